# revision 6
# baseline (speedup 1.0000x reference)
"""Conv2D (VALID, 3x3, NCHW) via 1D Winograd F(2,3) along W, on 8 TRN2 cores.

Problem: x (32,128,56,56) f32, weight (256,128,3,3) f32, bias (256,) f32
         -> out (32,256,54,54) f32.

Strategy:
  - Data-parallel over batch: 4 images per core, no collectives.
  - 1D Winograd F(2,3) along W: for output col pair (2j, 2j+1),
      d0..d3 = x[.., 2j..2j+3]
      V1 = d1+d2, V2 = d2-d1, V0 = d0-d2, V3 = d1-d3      (device, DVE/GpSimd)
      Wq = G-transformed weights along kw (host); W3 negated so PSUM
      bank 3 accumulates -M3.
      M_q = sum_kh sum_cin Wq[kh] * Vq(rows shifted by kh)  (PE, PSUM accum)
      y_even = M0+M1+M2 ; y_odd = M1-M2-M3   (bias added on HOST in f32)
    PE cols drop 1.5x vs direct conv: 288 matmuls x 486 cols per core.
  - Output combine per chunk: ACT evacuates M1,M2 to fp16 SBUF; DVE does
    s=m1+m2, d=m1-m2 (fp16 2x mode) then ot0=M0+s, ot1=(-M3)+d. 1888ns
    DVE + 1096ns ACT per 2430ns chunk -> PE is the only bottleneck.
  - Input DMAs spread over 4 queues (~100GB/s per-queue packet rate):
    x0 on scalar(+sync mid rows), x1 split gpsimd/tensor-ring, x2 gpsimd,
    x3 sync. V transforms for imgs 1-3 interleaved into the chunk loop
    (DVE FIFO never blocks on late input data); V3 planes on GpSimd.
  - Output written bf16 (halves store traffic); host upcasts to f32 and
    adds bias. Out-DMAs alternate sync/scalar; last chunk 4-way split.
"""

import numpy as np
import ml_dtypes

import concourse.bass as bass
import concourse.mybir as mybir
from concourse import bacc
import concourse.tile as tile
from concourse.tile import add_dep_helper
from concourse.bass_utils import run_bass_kernel_spmd

N, CIN, H, W = 32, 128, 56, 56
COUT, KH, KW = 256, 3, 3
HO, WO = H - KH + 1, W - KW + 1  # 54, 54
NCORES = 8
NPER = N // NCORES   # 4
CTILES = COUT // 128  # 2
JT = WO // 2          # 27 tiles along W
RCH = 18              # output rows per chunk
NCH = HO // RCH       # 3 chunks per (img, ctile)
NPIX = RCH * JT       # 486 <= 512 (one fp32 PSUM bank)
WCOLS = CTILES * 4 * KH * 128  # 3072
NWARM = 72

BF16 = mybir.dt.bfloat16
F16 = mybir.dt.float16
F32 = mybir.dt.float32

QORDER = (1, 2, 0, 3)  # M1/M2 finish first so ACT evac starts mid-chunk


def build_nc() -> bass.Bass:
    nc = bacc.Bacc(None)
    x_h = nc.dram_tensor("x", [NPER, CIN, 4, H * JT], BF16, kind="ExternalInput")
    w_h = nc.dram_tensor("w", [CIN, WCOLS], BF16, kind="ExternalInput")
    o_h = nc.dram_tensor("out", [NPER, COUT, 2, HO * JT], BF16, kind="ExternalOutput")

    with tile.TileContext(nc) as tc:
        with (
            tc.tile_pool(name="wpool", bufs=1) as wpool,
            tc.tile_pool(name="xpool", bufs=4) as xpool,
            tc.tile_pool(name="vpool", bufs=4) as vpool,
            tc.tile_pool(name="tpool", bufs=4) as tpool,
            tc.tile_pool(name="opool", bufs=4) as opool,
            tc.tile_pool(name="psum", bufs=8, space="PSUM") as psum_pool,
        ):
            # PE warmup for HAM un-throttle during the input-DMA window.
            wu = wpool.tile([CIN, 64], BF16)
            nc.gpsimd.memset(wu[:], 0)
            wupt = psum_pool.tile([32, 64], F32, tag="pt")
            warmups = []
            for _ in range(NWARM):
                warmups.append(
                    nc.tensor.matmul(wupt[:], wu[:, :32], wu[:, :64], start=True, stop=True)
                )

            # ---- input DMAs, spread across the 3 DMA-capable queues ----
            # (HWDGE per-queue packet rate ~100GB/s; gpsimd SWDGE takes the
            # latency-insensitive x2/x3.)
            xts = []
            for n in range(NPER):
                xts.append(xpool.tile([CIN, 4, H * JT], BF16, tag="xt", name=f"xt{n}"))
            # scalar: x0 rows 0-20 (chunk hc0), rows 20-38 (hc1), x1 planes 0,3
            x0a = [
                nc.scalar.dma_start(out=xts[0][:, q, 0 : 20 * JT], in_=x_h[0, :, q, 0 : 20 * JT])
                for q in range(4)
            ]
            # x0 rows 20-56: planes 1,2 (consumed first) on scalar behind x0a;
            # planes 0,3 on sync behind w1 — each lands ~1us before its chunk.
            x0b = [
                nc.scalar.dma_start(
                    out=xts[0][:, q, 20 * JT : 38 * JT], in_=x_h[0, :, q, 20 * JT : 38 * JT]
                )
                for q in (1, 2)
            ]
            for d in x0b:
                add_dep_helper(d.ins, warmups[10].ins, reason="defer x0b")
            x0c12 = [
                nc.scalar.dma_start(
                    out=xts[0][:, q, 38 * JT : 56 * JT], in_=x_h[0, :, q, 38 * JT : 56 * JT]
                )
                for q in (1, 2)
            ]
            x1p0 = nc.scalar.dma_start(out=xts[1][:, 0], in_=x_h[1, :, 0])
            add_dep_helper(x1p0.ins, warmups[50].ins, reason="defer x1p0")
            nc.scalar.dma_start(out=xts[1][:, 3], in_=x_h[1, :, 3])
            # sync: weights ct0, x0 rows 20-56 planes 0,3, weights ct1, x1 planes 1,2
            wt = wpool.tile([CIN, WCOLS], BF16)
            nc.sync.dma_start(out=wt[:, : WCOLS // 2], in_=w_h[:, : WCOLS // 2])
            for q in (0, 3):
                nc.sync.dma_start(
                    out=xts[0][:, q, 20 * JT : 56 * JT], in_=x_h[0, :, q, 20 * JT : 56 * JT]
                )
            w2 = nc.sync.dma_start(out=wt[:, WCOLS // 2 :], in_=w_h[:, WCOLS // 2 :])
            add_dep_helper(w2.ins, warmups[30].ins, reason="defer w2")
            x1p12 = nc.sync.dma_start(out=xts[1][:, 1:3], in_=x_h[1, :, 1:3])
            add_dep_helper(x1p12.ins, warmups[50].ins, reason="defer x1p12")
            # gpsimd (SWDGE): x2 whole, x3 whole
            x2d = nc.gpsimd.dma_start(out=xts[2][:], in_=x_h[2])
            add_dep_helper(x2d.ins, warmups[50].ins, reason="defer x2")
            x3d = nc.gpsimd.dma_start(out=xts[3][:], in_=x_h[3])

            # ---- V transform: V1=B+C, V2=C-B, V0=A-C, V3=B-D (planes 0..3) ----
            vts = []
            for n in range(NPER):
                vts.append(vpool.tile([CIN, 4, H * JT], BF16, tag="vt", name=f"vt{n}"))

            def vplane(eng, n, q, r0, r1):
                xt, vt = xts[n], vts[n]
                a, b = r0 * JT, r1 * JT
                if q == 0:
                    eng.tensor_sub(vt[:, 0, a:b], xt[:, 0, a:b], xt[:, 2, a:b])
                elif q == 1:
                    eng.tensor_add(vt[:, 1, a:b], xt[:, 1, a:b], xt[:, 2, a:b])
                elif q == 2:
                    eng.tensor_sub(vt[:, 2, a:b], xt[:, 2, a:b], xt[:, 1, a:b])
                else:
                    eng.tensor_sub(vt[:, 3, a:b], xt[:, 1, a:b], xt[:, 3, a:b])

            # img0 on DVE, split by rows to chase the x0 DMA pieces;
            # plane order matches QORDER consumption (V1 first).
            for r0, r1 in ((0, 20), (20, 38), (38, 56)):
                for q in (1, 2, 0, 3):
                    vplane(nc.vector, 0, q, r0, r1)
            # V3 planes for imgs 1-3 on GpSimd (queued behind its x DMAs).
            for n in range(1, NPER):
                vplane(nc.gpsimd, n, 3, 0, 56)
            # DVE planes (V1,V2,V0) for imgs 1-3 are interleaved into the
            # chunk loop below, ~1.5 chunks before first use (so a late x
            # DMA can only stall the DVE FIFO briefly).
            dve_v = {}
            for n in range(1, NPER):
                base = n * 2 * NCH  # img n's chunks start here
                dve_v.setdefault(base - 2, []).extend([(n, 1), (n, 2)])
                dve_v.setdefault(base - 1, []).append((n, 0))

            deferred = {6: [x3d]}
            mm_idx = 0
            chunk_id = 0
            nchunks = NPER * CTILES * NCH

            for n in range(NPER):
                for c in range(CTILES):
                    for hc in range(NCH):
                        h0 = hc * RCH
                        pts = {}
                        for q in QORDER:
                            pts[q] = psum_pool.tile(
                                [128, NPIX], F32, tag="pt", name=f"pt{n}_{c}_{hc}_{q}"
                            )
                        for q in QORDER:
                            for kh in range(KH):
                                off = ((c * 4 + q) * KH + kh) * 128
                                mm = nc.tensor.matmul(
                                    pts[q][:],
                                    wt[:, off : off + 128],
                                    vts[n][:, q, (h0 + kh) * JT : (h0 + kh + RCH) * JT],
                                    start=(kh == 0),
                                    stop=(kh == KH - 1),
                                )
                                for dma in deferred.get(mm_idx, ()):
                                    add_dep_helper(dma.ins, mm.ins, reason="defer DMA")
                                mm_idx += 1
                        # Evac: ACT copies M1,M2 to fp16 SBUF (frees those
                        # banks early); DVE does s/d in fp16 2x mode, then
                        # one PSUM-operand add per output plane.
                        ot = opool.tile([128, 2, NPIX], BF16, tag="ot")
                        m1 = tpool.tile([128, NPIX], F16, tag="m1")
                        m2 = tpool.tile([128, NPIX], F16, tag="m2")
                        s = tpool.tile([128, NPIX], F16, tag="s")
                        dd = tpool.tile([128, NPIX], F16, tag="d")
                        nc.scalar.copy(m1[:], pts[1][:])
                        nc.scalar.copy(m2[:], pts[2][:])
                        nc.vector.tensor_add(s[:], m1[:], m2[:])
                        nc.vector.tensor_sub(dd[:], m1[:], m2[:])
                        nc.vector.tensor_add(ot[:, 0], pts[0][:], s[:])
                        nc.vector.tensor_add(ot[:, 1], pts[3][:], dd[:])
                        co = c * 128
                        ha, hb = h0 * JT, (h0 + RCH) * JT
                        last = chunk_id == nchunks - 1
                        if not last:
                            ring = nc.sync if chunk_id % 2 == 0 else nc.scalar
                            ring.dma_start(out=o_h[n, co : co + 128, :, ha:hb], in_=ot[:])
                        else:
                            # drain the final chunk on 3 rings in parallel
                            hm = (ha + hb) // 2
                            nc.sync.dma_start(
                                out=o_h[n, co : co + 128, 0, ha:hb], in_=ot[:, 0]
                            )
                            nc.scalar.dma_start(
                                out=o_h[n, co : co + 128, 1, ha:hm], in_=ot[:, 1, : hm - ha]
                            )
                            nc.gpsimd.dma_start(
                                out=o_h[n, co : co + 128, 1, hm:hb], in_=ot[:, 1, hm - ha :]
                            )
                        for nn, q in dve_v.get(chunk_id, ()):
                            vplane(nc.vector, nn, q, 0, 56)
                        chunk_id += 1
    nc.finalize()
    return nc


_NC_CACHE = None


def _get_nc():
    global _NC_CACHE
    if _NC_CACHE is None:
        _NC_CACHE = build_nc()
    return _NC_CACHE


def _prep_in_maps(x, weight):
    bf16 = ml_dtypes.bfloat16
    w = weight.astype(np.float32)
    g0, g1, g2 = w[:, :, :, 0], w[:, :, :, 1], w[:, :, :, 2]  # [COUT, CIN, KH]
    # q3 negated: PSUM bank 3 accumulates -M3 so y_odd = M1-M2+(bank3)
    Wq = np.stack([g0, (g0 + g1 + g2) * 0.5, (g0 - g1 + g2) * 0.5, -g2], axis=0)
    # layout [CIN, ct, q, kh, m] -> [CIN, 3072]
    Wt = np.zeros((CIN, CTILES, 4, KH, 128), np.float32)
    for ct in range(CTILES):
        Wt[:, ct] = Wq[:, ct * 128 : (ct + 1) * 128].transpose(2, 0, 3, 1)
    w_t = np.ascontiguousarray(Wt.reshape(CIN, WCOLS)).astype(bf16)
    # x planes: A=x[0::2](27), B=x[1::2](27), C=x[2::2](27), D=x[3::2](27)
    P = np.stack(
        [x[:, :, :, 0:54:2], x[:, :, :, 1:55:2], x[:, :, :, 2:56:2], x[:, :, :, 3:56:2]],
        axis=2,
    ).reshape(N, CIN, 4, H * JT)  # [N, CIN, 4, H*27]
    in_maps = []
    for i in range(NCORES):
        xs = np.ascontiguousarray(P[i * NPER : (i + 1) * NPER]).astype(bf16)
        in_maps.append({"x": xs, "w": w_t})
    return in_maps


def run(x, weight, bias, trace=False):
    nc = _get_nc()
    in_maps = _prep_in_maps(x, weight)
    res = run_bass_kernel_spmd(nc, in_maps, core_ids=list(range(NCORES)), trace=trace)
    o = np.concatenate([r["out"] for r in res.results], axis=0).reshape(
        N, COUT, 2, HO, JT
    )
    out = np.empty((N, COUT, HO, WO), np.float32)
    out[:, :, :, 0::2] = o[:, :, 0].astype(np.float32)
    out[:, :, :, 1::2] = o[:, :, 1].astype(np.float32)
    out += np.asarray(bias, np.float32)[None, :, None, None]
    return out, res


def kernel(x: np.ndarray, weight: np.ndarray, bias: np.ndarray) -> np.ndarray:
    out, _ = run(x, weight, bias, trace=False)
    return out.astype(np.float32)


# revision 8
# speedup vs baseline: 1.1313x; 1.1313x over previous
"""Conv2D (VALID, 3x3, NCHW) via 1D Winograd F(2,3) along W, on 8 TRN2 cores.

Problem: x (32,128,56,56) f32, weight (256,128,3,3) f32, bias (256,) f32
         -> out (32,256,54,54) f32.

Strategy:
  - Data-parallel over batch: 4 images per core, no collectives.
  - 1D Winograd F(2,3) along W: for output col pair (2j, 2j+1),
      d0..d3 = x[.., 2j..2j+3]
      V1 = d1+d2, V2 = d2-d1, V0 = d0-d2, V3 = d1-d3      (device, DVE/GpSimd)
      Wq = G-transformed weights along kw (host); W3 negated so PSUM
      bank 3 accumulates -M3.
      M_q = sum_kh sum_cin Wq[kh] * Vq(rows shifted by kh)  (PE, PSUM accum)
      y_even = M0+M1+M2 ; y_odd = M1-M2-M3   (bias added on HOST in f32)
    PE cols drop 1.5x vs direct conv: 288 matmuls x 486 cols per core.
  - Output combine per chunk: ACT evacuates M1,M2 to fp16 SBUF; DVE does
    s=m1+m2, d=m1-m2 (fp16 2x mode) then ot0=M0+s, ot1=(-M3)+d. 1888ns
    DVE + 1096ns ACT per 2430ns chunk -> PE is the only bottleneck.
  - Input DMAs spread over 4 queues (~100GB/s per-queue packet rate):
    x0 on scalar(+sync mid rows), x1 split gpsimd/tensor-ring, x2 gpsimd,
    x3 sync. V transforms for imgs 1-3 interleaved into the chunk loop
    (DVE FIFO never blocks on late input data); V3 planes on GpSimd.
  - Output written bf16 (halves store traffic); host upcasts to f32 and
    adds bias. Out-DMAs alternate sync/scalar; last chunk 4-way split.
"""

import numpy as np
import ml_dtypes

import concourse.bass as bass
import concourse.mybir as mybir
from concourse import bacc
import concourse.tile as tile
from concourse.tile import add_dep_helper
from concourse.bass_utils import run_bass_kernel_spmd

N, CIN, H, W = 32, 128, 56, 56
COUT, KH, KW = 256, 3, 3
HO, WO = H - KH + 1, W - KW + 1  # 54, 54
NCORES = 8
NPER = N // NCORES   # 4
CTILES = COUT // 128  # 2
JT = WO // 2          # 27 tiles along W
RCH = 18              # output rows per chunk
NCH = HO // RCH       # 3 chunks per (img, ctile)
NPIX = RCH * JT       # 486 <= 512 (one fp32 PSUM bank)
WCOLS = CTILES * 4 * KH * 128  # 3072
NWARM = 72

BF16 = mybir.dt.bfloat16
F16 = mybir.dt.float16
F32 = mybir.dt.float32

QORDER = (1, 2, 0, 3)  # M1/M2 finish first so ACT evac starts mid-chunk


def build_nc() -> bass.Bass:
    nc = bacc.Bacc(None)
    x_h = nc.dram_tensor("x", [NPER, CIN, 4, H * JT], BF16, kind="ExternalInput")
    w_h = nc.dram_tensor("w", [CIN, WCOLS], BF16, kind="ExternalInput")
    o_h = nc.dram_tensor("out", [NPER, COUT, 2, HO * JT], BF16, kind="ExternalOutput")

    with tile.TileContext(nc) as tc:
        with (
            tc.tile_pool(name="wpool", bufs=1) as wpool,
            tc.tile_pool(name="xpool", bufs=4) as xpool,
            tc.tile_pool(name="vpool", bufs=4) as vpool,
            tc.tile_pool(name="tpool", bufs=4) as tpool,
            tc.tile_pool(name="opool", bufs=4) as opool,
            tc.tile_pool(name="psum", bufs=8, space="PSUM") as psum_pool,
        ):
            # PE warmup for HAM un-throttle during the input-DMA window.
            wu = wpool.tile([CIN, 64], BF16)
            nc.gpsimd.memset(wu[:], 0)
            wupt = psum_pool.tile([32, 64], F32, tag="pt")
            warmups = []
            for _ in range(NWARM):
                warmups.append(
                    nc.tensor.matmul(wupt[:], wu[:, :32], wu[:, :64], start=True, stop=True)
                )

            # ---- input DMAs, spread across the 3 DMA-capable queues ----
            # (HWDGE per-queue packet rate ~100GB/s; gpsimd SWDGE takes the
            # latency-insensitive x2/x3.)
            xts = []
            for n in range(NPER):
                xts.append(xpool.tile([CIN, 4, H * JT], BF16, tag="xt", name=f"xt{n}"))
            # scalar: x0 rows 0-20 (chunk hc0), rows 20-38 (hc1), x1 planes 0,3
            x0a = [
                nc.scalar.dma_start(out=xts[0][:, q, 0 : 20 * JT], in_=x_h[0, :, q, 0 : 20 * JT])
                for q in range(4)
            ]
            # x0 rows 20-56: planes 1,2 (consumed first) on scalar behind x0a;
            # planes 0,3 on sync behind w1 — each lands ~1us before its chunk.
            x0b = [
                nc.scalar.dma_start(
                    out=xts[0][:, q, 20 * JT : 38 * JT], in_=x_h[0, :, q, 20 * JT : 38 * JT]
                )
                for q in (1, 2)
            ]
            for d in x0b:
                add_dep_helper(d.ins, warmups[10].ins, reason="defer x0b")
            x0c12 = [
                nc.scalar.dma_start(
                    out=xts[0][:, q, 38 * JT : 56 * JT], in_=x_h[0, :, q, 38 * JT : 56 * JT]
                )
                for q in (1, 2)
            ]
            # sync: weights ct0, x0 rows 20-56 planes 0,3, weights ct1
            wt = wpool.tile([CIN, WCOLS], BF16)
            nc.sync.dma_start(out=wt[:, : WCOLS // 2], in_=w_h[:, : WCOLS // 2])
            for q in (0, 3):
                nc.sync.dma_start(
                    out=xts[0][:, q, 20 * JT : 56 * JT], in_=x_h[0, :, q, 20 * JT : 56 * JT]
                )
            w2 = nc.sync.dma_start(out=wt[:, WCOLS // 2 :], in_=w_h[:, WCOLS // 2 :])
            add_dep_helper(w2.ins, warmups[30].ins, reason="defer w2")
            # gpsimd (SWDGE, ~180GB/s measured): x1, x2, x3 whole images.
            # NOTE: no compute on gpsimd — its SBUF port is DVE's shared
            # second read port, so any gpsimd op fully blocks DVE TTs.
            x1d = nc.gpsimd.dma_start(out=xts[1][:], in_=x_h[1])
            add_dep_helper(x1d.ins, warmups[50].ins, reason="defer x1")
            nc.gpsimd.dma_start(out=xts[2][:], in_=x_h[2])
            x3d = nc.gpsimd.dma_start(out=xts[3][:], in_=x_h[3])

            # ---- V transform: V1=B+C, V2=C-B, V0=A-C, V3=B-D (planes 0..3) ----
            vts = []
            for n in range(NPER):
                vts.append(vpool.tile([CIN, 4, H * JT], BF16, tag="vt", name=f"vt{n}"))

            def vplane(eng, n, q, r0, r1):
                xt, vt = xts[n], vts[n]
                a, b = r0 * JT, r1 * JT
                if q == 0:
                    eng.tensor_sub(vt[:, 0, a:b], xt[:, 0, a:b], xt[:, 2, a:b])
                elif q == 1:
                    eng.tensor_add(vt[:, 1, a:b], xt[:, 1, a:b], xt[:, 2, a:b])
                elif q == 2:
                    eng.tensor_sub(vt[:, 2, a:b], xt[:, 2, a:b], xt[:, 1, a:b])
                else:
                    eng.tensor_sub(vt[:, 3, a:b], xt[:, 1, a:b], xt[:, 3, a:b])

            # img0 on DVE, split by rows to chase the x0 DMA pieces;
            # plane order matches QORDER consumption (V1 first).
            for r0, r1 in ((0, 20), (20, 38), (38, 56)):
                for q in (1, 2, 0, 3):
                    vplane(nc.vector, 0, q, r0, r1)
            # All V planes for imgs 1-3 on DVE, interleaved into the chunk
            # loop below ~2-3 chunks before first use (so a late x DMA can
            # only stall the DVE FIFO briefly).
            dve_v = {}
            for n in range(1, NPER):
                base = n * 2 * NCH  # img n's chunks start here
                dve_v.setdefault(base - 3, []).extend([(n, 1), (n, 2)])
                dve_v.setdefault(base - 2, []).extend([(n, 0), (n, 3)])

            deferred = {6: [x3d]}
            mm_idx = 0
            chunk_id = 0
            nchunks = NPER * CTILES * NCH

            for n in range(NPER):
                for c in range(CTILES):
                    for hc in range(NCH):
                        h0 = hc * RCH
                        pts = {}
                        for q in QORDER:
                            pts[q] = psum_pool.tile(
                                [128, NPIX], F32, tag="pt", name=f"pt{n}_{c}_{hc}_{q}"
                            )
                        for q in QORDER:
                            for kh in range(KH):
                                off = ((c * 4 + q) * KH + kh) * 128
                                mm = nc.tensor.matmul(
                                    pts[q][:],
                                    wt[:, off : off + 128],
                                    vts[n][:, q, (h0 + kh) * JT : (h0 + kh + RCH) * JT],
                                    start=(kh == 0),
                                    stop=(kh == KH - 1),
                                )
                                for dma in deferred.get(mm_idx, ()):
                                    add_dep_helper(dma.ins, mm.ins, reason="defer DMA")
                                mm_idx += 1
                        # Evac: ACT copies M1,M2 to fp16 SBUF (frees those
                        # banks early); DVE does s/d in fp16 2x mode, then
                        # one PSUM-operand add per output plane.
                        ot = opool.tile([128, 2, NPIX], BF16, tag="ot")
                        m1 = tpool.tile([128, NPIX], F16, tag="m1")
                        m2 = tpool.tile([128, NPIX], F16, tag="m2")
                        s = tpool.tile([128, NPIX], F16, tag="s")
                        dd = tpool.tile([128, NPIX], F16, tag="d")
                        nc.scalar.copy(m1[:], pts[1][:])
                        nc.scalar.copy(m2[:], pts[2][:])
                        nc.vector.tensor_add(s[:], m1[:], m2[:])
                        nc.vector.tensor_sub(dd[:], m1[:], m2[:])
                        nc.vector.tensor_add(ot[:, 0], pts[0][:], s[:])
                        nc.vector.tensor_add(ot[:, 1], pts[3][:], dd[:])
                        co = c * 128
                        ha, hb = h0 * JT, (h0 + RCH) * JT
                        last = chunk_id == nchunks - 1
                        if not last:
                            ring = nc.sync if chunk_id % 2 == 0 else nc.scalar
                            ring.dma_start(out=o_h[n, co : co + 128, :, ha:hb], in_=ot[:])
                        else:
                            # drain the final chunk on 3 rings in parallel
                            hm = (ha + hb) // 2
                            nc.sync.dma_start(
                                out=o_h[n, co : co + 128, 0, ha:hb], in_=ot[:, 0]
                            )
                            nc.scalar.dma_start(
                                out=o_h[n, co : co + 128, 1, ha:hm], in_=ot[:, 1, : hm - ha]
                            )
                            nc.gpsimd.dma_start(
                                out=o_h[n, co : co + 128, 1, hm:hb], in_=ot[:, 1, hm - ha :]
                            )
                        for nn, q in dve_v.get(chunk_id, ()):
                            vplane(nc.vector, nn, q, 0, 56)
                        chunk_id += 1
    nc.finalize()
    return nc


_NC_CACHE = None


def _get_nc():
    global _NC_CACHE
    if _NC_CACHE is None:
        _NC_CACHE = build_nc()
    return _NC_CACHE


def _prep_in_maps(x, weight):
    bf16 = ml_dtypes.bfloat16
    w = weight.astype(np.float32)
    g0, g1, g2 = w[:, :, :, 0], w[:, :, :, 1], w[:, :, :, 2]  # [COUT, CIN, KH]
    # q3 negated: PSUM bank 3 accumulates -M3 so y_odd = M1-M2+(bank3)
    Wq = np.stack([g0, (g0 + g1 + g2) * 0.5, (g0 - g1 + g2) * 0.5, -g2], axis=0)
    # layout [CIN, ct, q, kh, m] -> [CIN, 3072]
    Wt = np.zeros((CIN, CTILES, 4, KH, 128), np.float32)
    for ct in range(CTILES):
        Wt[:, ct] = Wq[:, ct * 128 : (ct + 1) * 128].transpose(2, 0, 3, 1)
    w_t = np.ascontiguousarray(Wt.reshape(CIN, WCOLS)).astype(bf16)
    # x planes: A=x[0::2](27), B=x[1::2](27), C=x[2::2](27), D=x[3::2](27)
    P = np.stack(
        [x[:, :, :, 0:54:2], x[:, :, :, 1:55:2], x[:, :, :, 2:56:2], x[:, :, :, 3:56:2]],
        axis=2,
    ).reshape(N, CIN, 4, H * JT)  # [N, CIN, 4, H*27]
    in_maps = []
    for i in range(NCORES):
        xs = np.ascontiguousarray(P[i * NPER : (i + 1) * NPER]).astype(bf16)
        in_maps.append({"x": xs, "w": w_t})
    return in_maps


def run(x, weight, bias, trace=False):
    nc = _get_nc()
    in_maps = _prep_in_maps(x, weight)
    res = run_bass_kernel_spmd(nc, in_maps, core_ids=list(range(NCORES)), trace=trace)
    o = np.concatenate([r["out"] for r in res.results], axis=0).reshape(
        N, COUT, 2, HO, JT
    )
    out = np.empty((N, COUT, HO, WO), np.float32)
    out[:, :, :, 0::2] = o[:, :, 0].astype(np.float32)
    out[:, :, :, 1::2] = o[:, :, 1].astype(np.float32)
    out += np.asarray(bias, np.float32)[None, :, None, None]
    return out, res


def kernel(x: np.ndarray, weight: np.ndarray, bias: np.ndarray) -> np.ndarray:
    out, _ = run(x, weight, bias, trace=False)
    return out.astype(np.float32)


# revision 12
# speedup vs baseline: 1.1370x; 1.0050x over previous
"""Conv2D (VALID, 3x3, NCHW) via 1D Winograd F(2,3) along W, on 8 TRN2 cores.

Problem: x (32,128,56,56) f32, weight (256,128,3,3) f32, bias (256,) f32
         -> out (32,256,54,54) f32.

Strategy:
  - Data-parallel over batch: 4 images per core, no collectives.
  - 1D Winograd F(2,3) along W: for output col pair (2j, 2j+1),
      d0..d3 = x[.., 2j..2j+3]
      V1 = d1+d2, V2 = d2-d1, V0 = d0-d2, V3 = d1-d3      (device, DVE/GpSimd)
      Wq = G-transformed weights along kw (host); W3 negated so PSUM
      bank 3 accumulates -M3.
      M_q = sum_kh sum_cin Wq[kh] * Vq(rows shifted by kh)  (PE, PSUM accum)
      y_even = M0+M1+M2 ; y_odd = M1-M2-M3   (bias added on HOST in f32)
    PE cols drop 1.5x vs direct conv: 288 matmuls x 486 cols per core.
  - Output combine per chunk: ACT evacuates M1,M2 to fp16 SBUF; DVE does
    s=m1+m2, d=m1-m2 (fp16 2x mode) then ot0=M0+s, ot1=(-M3)+d. 1888ns
    DVE + 1096ns ACT per 2430ns chunk -> PE is the only bottleneck.
  - Input DMAs spread over 4 queues (~100GB/s per-queue packet rate):
    x0 on scalar(+sync mid rows), x1 split gpsimd/tensor-ring, x2 gpsimd,
    x3 sync. V transforms for imgs 1-3 interleaved into the chunk loop
    (DVE FIFO never blocks on late input data); V3 planes on GpSimd.
  - Output written bf16 (halves store traffic); host upcasts to f32 and
    adds bias. Out-DMAs alternate sync/scalar; last chunk 4-way split.
"""

import numpy as np
import ml_dtypes

import concourse.bass as bass
import concourse.mybir as mybir
from concourse import bacc
import concourse.tile as tile
from concourse.tile import add_dep_helper
from concourse.bass_utils import run_bass_kernel_spmd

N, CIN, H, W = 32, 128, 56, 56
COUT, KH, KW = 256, 3, 3
HO, WO = H - KH + 1, W - KW + 1  # 54, 54
NCORES = 8
NPER = N // NCORES   # 4
CTILES = COUT // 128  # 2
JT = WO // 2          # 27 tiles along W
RCH = 18              # output rows per chunk
NCH = HO // RCH       # 3 chunks per (img, ctile)
NPIX = RCH * JT       # 486 <= 512 (one fp32 PSUM bank)
WCOLS = CTILES * 4 * KH * 128  # 3072
NWARM = 64

BF16 = mybir.dt.bfloat16
F16 = mybir.dt.float16
F32 = mybir.dt.float32

QORDER = (1, 2, 0, 3)  # M1/M2 finish first so ACT evac starts mid-chunk


def build_nc() -> bass.Bass:
    nc = bacc.Bacc(None)
    x_h = nc.dram_tensor("x", [NPER, CIN, 4, H * JT], BF16, kind="ExternalInput")
    w_h = nc.dram_tensor("w", [CIN, WCOLS], BF16, kind="ExternalInput")
    o_h = nc.dram_tensor("out", [NPER, COUT, 2, HO * JT], BF16, kind="ExternalOutput")

    with tile.TileContext(nc) as tc:
        with (
            tc.tile_pool(name="wpool", bufs=1) as wpool,
            tc.tile_pool(name="xpool", bufs=4) as xpool,
            tc.tile_pool(name="vpool", bufs=4) as vpool,
            tc.tile_pool(name="tpool", bufs=4) as tpool,
            tc.tile_pool(name="opool", bufs=4) as opool,
            tc.tile_pool(name="psum", bufs=8, space="PSUM") as psum_pool,
        ):
            # PE warmup for HAM un-throttle during the input-DMA window.
            # memset on vector: its preamble ends earliest and gpsimd memset
            # was observed to delay the first warmup to ~7.4us.
            wu = wpool.tile([CIN, 64], BF16)
            nc.vector.memset(wu[:], 0)
            wupt = psum_pool.tile([32, 64], F32, tag="pt")
            warmups = []
            for _ in range(NWARM):
                warmups.append(
                    nc.tensor.matmul(wupt[:], wu[:, :32], wu[:, :64], start=True, stop=True)
                )

            # ---- input DMAs, spread across the 3 DMA-capable queues ----
            # (HWDGE per-queue packet rate ~100GB/s; gpsimd SWDGE takes the
            # latency-insensitive x2/x3.)
            xts = []
            for n in range(NPER):
                xts.append(xpool.tile([CIN, 4, H * JT], BF16, tag="xt", name=f"xt{n}"))
            # scalar: x0 planes 1,2 (consumed first per chunk) by row range
            x0a = [
                nc.scalar.dma_start(out=xts[0][:, q, 0 : 20 * JT], in_=x_h[0, :, q, 0 : 20 * JT])
                for q in (1, 2)
            ]
            x0b = [
                nc.scalar.dma_start(
                    out=xts[0][:, q, 20 * JT : 38 * JT], in_=x_h[0, :, q, 20 * JT : 38 * JT]
                )
                for q in (1, 2)
            ]
            for d in x0b:
                add_dep_helper(d.ins, warmups[10].ins, reason="defer x0b")
            x0c12 = [
                nc.scalar.dma_start(
                    out=xts[0][:, q, 38 * JT : 56 * JT], in_=x_h[0, :, q, 38 * JT : 56 * JT]
                )
                for q in (1, 2)
            ]
            # sync: weights ct0, then x0 planes 0,3 by row range, weights ct1
            wt = wpool.tile([CIN, WCOLS], BF16)
            nc.sync.dma_start(out=wt[:, : WCOLS // 2], in_=w_h[:, : WCOLS // 2])
            for r0, r1 in ((0, 20), (20, 38), (38, 56)):
                for q in (0, 3):
                    nc.sync.dma_start(
                        out=xts[0][:, q, r0 * JT : r1 * JT], in_=x_h[0, :, q, r0 * JT : r1 * JT]
                    )
            w2 = nc.sync.dma_start(out=wt[:, WCOLS // 2 :], in_=w_h[:, WCOLS // 2 :])
            add_dep_helper(w2.ins, warmups[30].ins, reason="defer w2")
            # gpsimd (SWDGE, ~180GB/s measured): x1, x2, x3 whole images.
            # NOTE: no compute on gpsimd — its SBUF port is DVE's shared
            # second read port, so any gpsimd op fully blocks DVE TTs.
            x1a = nc.gpsimd.dma_start(out=xts[1][:, 1:3], in_=x_h[1, :, 1:3])
            add_dep_helper(x1a.ins, warmups[30].ins, reason="defer x1")
            nc.gpsimd.dma_start(out=xts[1][:, 0], in_=x_h[1, :, 0])
            nc.gpsimd.dma_start(out=xts[1][:, 3], in_=x_h[1, :, 3])
            nc.gpsimd.dma_start(out=xts[2][:], in_=x_h[2])
            x3d = nc.gpsimd.dma_start(out=xts[3][:], in_=x_h[3])

            # ---- V transform: V1=B+C, V2=C-B, V0=A-C, V3=B-D (planes 0..3) ----
            vts = []
            for n in range(NPER):
                vts.append(vpool.tile([CIN, 4, H * JT], BF16, tag="vt", name=f"vt{n}"))

            def vplane(eng, n, q, r0, r1):
                xt, vt = xts[n], vts[n]
                a, b = r0 * JT, r1 * JT
                if q == 0:
                    eng.tensor_sub(vt[:, 0, a:b], xt[:, 0, a:b], xt[:, 2, a:b])
                elif q == 1:
                    eng.tensor_add(vt[:, 1, a:b], xt[:, 1, a:b], xt[:, 2, a:b])
                elif q == 2:
                    eng.tensor_sub(vt[:, 2, a:b], xt[:, 2, a:b], xt[:, 1, a:b])
                else:
                    eng.tensor_sub(vt[:, 3, a:b], xt[:, 1, a:b], xt[:, 3, a:b])

            # img0 on DVE, split by rows to chase the x0 DMA pieces;
            # plane order matches QORDER consumption (V1 first).
            for r0, r1 in ((0, 20), (20, 38), (38, 56)):
                for q in (1, 2, 0, 3):
                    vplane(nc.vector, 0, q, r0, r1)
            # All V planes for imgs 1-3 on DVE, interleaved into the chunk
            # loop below ~2-3 chunks before first use (so a late x DMA can
            # only stall the DVE FIFO briefly).
            dve_v = {}
            for n in range(1, NPER):
                base = n * 2 * NCH  # img n's chunks start here
                dve_v.setdefault(base - 3, []).extend([(n, 1), (n, 2)])
                dve_v.setdefault(base - 2, []).extend([(n, 0), (n, 3)])

            deferred = {6: [x3d]}
            mm_idx = 0
            chunk_id = 0
            nchunks = NPER * CTILES * NCH

            for n in range(NPER):
                for c in range(CTILES):
                    for hc in range(NCH):
                        h0 = hc * RCH
                        pts = {}
                        for q in QORDER:
                            pts[q] = psum_pool.tile(
                                [128, NPIX], F32, tag="pt", name=f"pt{n}_{c}_{hc}_{q}"
                            )
                        for q in QORDER:
                            for kh in range(KH):
                                off = ((c * 4 + q) * KH + kh) * 128
                                mm = nc.tensor.matmul(
                                    pts[q][:],
                                    wt[:, off : off + 128],
                                    vts[n][:, q, (h0 + kh) * JT : (h0 + kh + RCH) * JT],
                                    start=(kh == 0),
                                    stop=(kh == KH - 1),
                                )
                                for dma in deferred.get(mm_idx, ()):
                                    add_dep_helper(dma.ins, mm.ins, reason="defer DMA")
                                mm_idx += 1
                        # Evac: ACT copies M1,M2 to fp16 SBUF (frees those
                        # banks early); DVE does s/d in fp16 2x mode, then
                        # one PSUM-operand add per output plane.
                        ot = opool.tile([128, 2, NPIX], BF16, tag="ot")
                        m1 = tpool.tile([128, NPIX], F16, tag="m1")
                        m2 = tpool.tile([128, NPIX], F16, tag="m2")
                        s = tpool.tile([128, NPIX], F16, tag="s")
                        dd = tpool.tile([128, NPIX], F16, tag="d")
                        nc.scalar.copy(m1[:], pts[1][:])
                        nc.scalar.copy(m2[:], pts[2][:])
                        nc.vector.tensor_add(s[:], m1[:], m2[:])
                        nc.vector.tensor_sub(dd[:], m1[:], m2[:])
                        nc.vector.tensor_add(ot[:, 0], pts[0][:], s[:])
                        nc.vector.tensor_add(ot[:, 1], pts[3][:], dd[:])
                        co = c * 128
                        ha, hb = h0 * JT, (h0 + RCH) * JT
                        last = chunk_id == nchunks - 1
                        if not last:
                            if chunk_id < 18:
                                ring = (nc.sync, nc.scalar, nc.gpsimd)[chunk_id % 3]
                            else:
                                ring = nc.sync if chunk_id % 2 == 0 else nc.scalar
                            ring.dma_start(out=o_h[n, co : co + 128, :, ha:hb], in_=ot[:])
                        else:
                            # drain the final chunk on 3 rings in parallel
                            hm = (ha + hb) // 2
                            nc.sync.dma_start(
                                out=o_h[n, co : co + 128, 0, ha:hb], in_=ot[:, 0]
                            )
                            nc.scalar.dma_start(
                                out=o_h[n, co : co + 128, 1, ha:hm], in_=ot[:, 1, : hm - ha]
                            )
                            nc.gpsimd.dma_start(
                                out=o_h[n, co : co + 128, 1, hm:hb], in_=ot[:, 1, hm - ha :]
                            )
                        for nn, q in dve_v.get(chunk_id, ()):
                            vplane(nc.vector, nn, q, 0, 56)
                        chunk_id += 1
    nc.finalize()
    return nc


_NC_CACHE = None


def _get_nc():
    global _NC_CACHE
    if _NC_CACHE is None:
        _NC_CACHE = build_nc()
    return _NC_CACHE


def _prep_in_maps(x, weight):
    bf16 = ml_dtypes.bfloat16
    w = weight.astype(np.float32)
    g0, g1, g2 = w[:, :, :, 0], w[:, :, :, 1], w[:, :, :, 2]  # [COUT, CIN, KH]
    # q3 negated: PSUM bank 3 accumulates -M3 so y_odd = M1-M2+(bank3)
    Wq = np.stack([g0, (g0 + g1 + g2) * 0.5, (g0 - g1 + g2) * 0.5, -g2], axis=0)
    # layout [CIN, ct, q, kh, m] -> [CIN, 3072]
    Wt = np.zeros((CIN, CTILES, 4, KH, 128), np.float32)
    for ct in range(CTILES):
        Wt[:, ct] = Wq[:, ct * 128 : (ct + 1) * 128].transpose(2, 0, 3, 1)
    w_t = np.ascontiguousarray(Wt.reshape(CIN, WCOLS)).astype(bf16)
    # x planes: A=x[0::2](27), B=x[1::2](27), C=x[2::2](27), D=x[3::2](27)
    P = np.stack(
        [x[:, :, :, 0:54:2], x[:, :, :, 1:55:2], x[:, :, :, 2:56:2], x[:, :, :, 3:56:2]],
        axis=2,
    ).reshape(N, CIN, 4, H * JT)  # [N, CIN, 4, H*27]
    in_maps = []
    for i in range(NCORES):
        xs = np.ascontiguousarray(P[i * NPER : (i + 1) * NPER]).astype(bf16)
        in_maps.append({"x": xs, "w": w_t})
    return in_maps


def run(x, weight, bias, trace=False):
    nc = _get_nc()
    in_maps = _prep_in_maps(x, weight)
    res = run_bass_kernel_spmd(nc, in_maps, core_ids=list(range(NCORES)), trace=trace)
    o = np.concatenate([r["out"] for r in res.results], axis=0).reshape(
        N, COUT, 2, HO, JT
    )
    out = np.empty((N, COUT, HO, WO), np.float32)
    out[:, :, :, 0::2] = o[:, :, 0].astype(np.float32)
    out[:, :, :, 1::2] = o[:, :, 1].astype(np.float32)
    out += np.asarray(bias, np.float32)[None, :, None, None]
    return out, res


def kernel(x: np.ndarray, weight: np.ndarray, bias: np.ndarray) -> np.ndarray:
    out, _ = run(x, weight, bias, trace=False)
    return out.astype(np.float32)


# revision 16
# speedup vs baseline: 1.1536x; 1.0146x over previous
"""Conv2D (VALID, 3x3, NCHW) via 1D Winograd F(2,3) along W, on 8 TRN2 cores.

Problem: x (32,128,56,56) f32, weight (256,128,3,3) f32, bias (256,) f32
         -> out (32,256,54,54) f32.

Strategy:
  - Data-parallel over batch: 4 images per core, no collectives.
  - 1D Winograd F(2,3) along W: for output col pair (2j, 2j+1),
      d0..d3 = x[.., 2j..2j+3]
      V1 = d1+d2, V2 = d2-d1, V0 = d0-d2, V3 = d1-d3      (device, DVE/GpSimd)
      Wq = G-transformed weights along kw (host); W3 negated so PSUM
      bank 3 accumulates -M3.
      M_q = sum_kh sum_cin Wq[kh] * Vq(rows shifted by kh)  (PE, PSUM accum)
      y_even = M0+M1+M2 ; y_odd = M1-M2-M3   (bias added on HOST in f32)
    PE cols drop 1.5x vs direct conv: 288 matmuls x 486 cols per core.
  - Output combine per chunk: ACT evacuates M1,M2 to fp16 SBUF; DVE does
    s=m1+m2, d=m1-m2 (fp16 2x mode) then ot0=M0+s, ot1=(-M3)+d. 1888ns
    DVE + 1096ns ACT per 2430ns chunk -> PE is the only bottleneck.
  - Input DMAs spread over 4 queues (~100GB/s per-queue packet rate):
    x0 on scalar(+sync mid rows), x1 split gpsimd/tensor-ring, x2 gpsimd,
    x3 sync. V transforms for imgs 1-3 interleaved into the chunk loop
    (DVE FIFO never blocks on late input data); V3 planes on GpSimd.
  - Output written bf16 (halves store traffic); host upcasts to f32 and
    adds bias. Out-DMAs alternate sync/scalar; last chunk 4-way split.
"""

import numpy as np
import ml_dtypes

import concourse.bass as bass
import concourse.mybir as mybir
from concourse import bacc
import concourse.tile as tile
from concourse.tile import add_dep_helper
from concourse.bass_utils import run_bass_kernel_spmd

N, CIN, H, W = 32, 128, 56, 56
COUT, KH, KW = 256, 3, 3
HO, WO = H - KH + 1, W - KW + 1  # 54, 54
NCORES = 8
NPER = N // NCORES   # 4
CTILES = COUT // 128  # 2
JT = WO // 2          # 27 tiles along W
RCH = 18              # output rows per chunk
NCH = HO // RCH       # 3 chunks per (img, ctile)
NPIX = RCH * JT       # 486 <= 512 (one fp32 PSUM bank)
WCOLS = CTILES * 4 * KH * 128  # 3072
NWARM = 48

BF16 = mybir.dt.bfloat16
F16 = mybir.dt.float16
F32 = mybir.dt.float32

QORDER = (1, 2, 0, 3)  # M1/M2 finish first so ACT evac starts mid-chunk


def build_nc() -> bass.Bass:
    nc = bacc.Bacc(None)
    x_h = nc.dram_tensor("x", [NPER, CIN, 4, H * JT], BF16, kind="ExternalInput")
    w_h = nc.dram_tensor("w", [CIN, WCOLS], BF16, kind="ExternalInput")
    o_h = nc.dram_tensor("out", [NPER, COUT, 2, HO * JT], BF16, kind="ExternalOutput")

    with tile.TileContext(nc) as tc:
        with (
            tc.tile_pool(name="wpool", bufs=1) as wpool,
            tc.tile_pool(name="xpool", bufs=4) as xpool,
            tc.tile_pool(name="vpool", bufs=4) as vpool,
            tc.tile_pool(name="tpool", bufs=4) as tpool,
            tc.tile_pool(name="opool", bufs=4) as opool,
            tc.tile_pool(name="psum", bufs=8, space="PSUM") as psum_pool,
        ):
            # PE warmup for HAM un-throttle during the input-DMA window.
            # memset on vector: its preamble ends earliest and gpsimd memset
            # was observed to delay the first warmup to ~7.4us.
            wu = wpool.tile([CIN, 64], BF16)
            nc.vector.memset(wu[:], 0)
            wupt = psum_pool.tile([32, 64], F32, tag="pt")
            warmups = []
            for _ in range(NWARM):
                warmups.append(
                    nc.tensor.matmul(wupt[:], wu[:, :32], wu[:, :64], start=True, stop=True)
                )

            # ---- input DMAs, spread across the 3 DMA-capable queues ----
            # (HWDGE per-queue packet rate ~100GB/s; gpsimd SWDGE takes the
            # latency-insensitive x2/x3.)
            xts = []
            for n in range(NPER):
                xts.append(xpool.tile([CIN, 4, H * JT], BF16, tag="xt", name=f"xt{n}"))
            # Each queue transfers strictly FIFO, so emission order IS the
            # priority; no deferral anchors needed. Trigger->first-packet
            # latency is ~4-5us, so everything early must be small.
            # scalar: x0 planes 1,2 (consumed first per chunk), then w ct1
            x0a = [
                nc.scalar.dma_start(out=xts[0][:, q, 0 : 20 * JT], in_=x_h[0, :, q, 0 : 20 * JT])
                for q in (1, 2)
            ]
            x0b = [
                nc.scalar.dma_start(
                    out=xts[0][:, q, 20 * JT : 38 * JT], in_=x_h[0, :, q, 20 * JT : 38 * JT]
                )
                for q in (1, 2)
            ]
            wt = wpool.tile([CIN, WCOLS], BF16)
            w2 = nc.scalar.dma_start(out=wt[:, WCOLS // 2 :], in_=w_h[:, WCOLS // 2 :])
            # sync: weights ct0, then x0 planes 0,3 by row range
            nc.sync.dma_start(out=wt[:, : WCOLS // 2], in_=w_h[:, : WCOLS // 2])
            for r0, r1 in ((0, 20), (20, 38), (38, 56)):
                for q in (0, 3):
                    nc.sync.dma_start(
                        out=xts[0][:, q, r0 * JT : r1 * JT], in_=x_h[0, :, q, r0 * JT : r1 * JT]
                    )
            # gpsimd (SWDGE, ~260GB/s measured): x0 rows 38-56 planes 1,2,
            # then x1, x2, x3 whole images.
            # NOTE: no compute on gpsimd — its SBUF port is DVE's shared
            # second read port, so any gpsimd op fully blocks DVE TTs.
            for q in (1, 2):
                nc.gpsimd.dma_start(
                    out=xts[0][:, q, 38 * JT : 56 * JT], in_=x_h[0, :, q, 38 * JT : 56 * JT]
                )
            nc.gpsimd.dma_start(out=xts[1][:, 1:3], in_=x_h[1, :, 1:3])
            nc.gpsimd.dma_start(out=xts[1][:, 0], in_=x_h[1, :, 0])
            nc.gpsimd.dma_start(out=xts[1][:, 3], in_=x_h[1, :, 3])
            nc.gpsimd.dma_start(out=xts[2][:], in_=x_h[2])
            nc.gpsimd.dma_start(out=xts[3][:], in_=x_h[3])

            # ---- V transform: V1=B+C, V2=C-B, V0=A-C, V3=B-D (planes 0..3) ----
            vts = []
            for n in range(NPER):
                vts.append(vpool.tile([CIN, 4, H * JT], BF16, tag="vt", name=f"vt{n}"))

            def vplane(eng, n, q, r0, r1):
                xt, vt = xts[n], vts[n]
                a, b = r0 * JT, r1 * JT
                if q == 0:
                    eng.tensor_sub(vt[:, 0, a:b], xt[:, 0, a:b], xt[:, 2, a:b])
                elif q == 1:
                    eng.tensor_add(vt[:, 1, a:b], xt[:, 1, a:b], xt[:, 2, a:b])
                elif q == 2:
                    eng.tensor_sub(vt[:, 2, a:b], xt[:, 2, a:b], xt[:, 1, a:b])
                else:
                    eng.tensor_sub(vt[:, 3, a:b], xt[:, 1, a:b], xt[:, 3, a:b])

            # img0 on DVE, split by rows to chase the x0 DMA pieces;
            # plane order matches QORDER consumption (V1 first).
            for r0, r1 in ((0, 20), (20, 38), (38, 56)):
                for q in (1, 2, 0, 3):
                    vplane(nc.vector, 0, q, r0, r1)
            # All V planes for imgs 1-3 on DVE, interleaved into the chunk
            # loop below ~2-3 chunks before first use (so a late x DMA can
            # only stall the DVE FIFO briefly).
            dve_v = {}
            for n in range(1, NPER):
                base = n * 2 * NCH  # img n's chunks start here
                dve_v.setdefault(base - 3, []).extend([(n, 1), (n, 2)])
                dve_v.setdefault(base - 2, []).extend([(n, 0), (n, 3)])

            mm_idx = 0
            chunk_id = 0
            nchunks = NPER * CTILES * NCH

            for n in range(NPER):
                for c in range(CTILES):
                    for hc in range(NCH):
                        h0 = hc * RCH
                        pts = {}
                        for q in QORDER:
                            pts[q] = psum_pool.tile(
                                [128, NPIX], F32, tag="pt", name=f"pt{n}_{c}_{hc}_{q}"
                            )
                        for q in QORDER:
                            for kh in range(KH):
                                off = ((c * 4 + q) * KH + kh) * 128
                                nc.tensor.matmul(
                                    pts[q][:],
                                    wt[:, off : off + 128],
                                    vts[n][:, q, (h0 + kh) * JT : (h0 + kh + RCH) * JT],
                                    start=(kh == 0),
                                    stop=(kh == KH - 1),
                                )
                                mm_idx += 1
                        # Evac: ACT copies M1,M2 to fp16 SBUF (frees those
                        # banks early); DVE does s/d in fp16 2x mode, then
                        # one PSUM-operand add per output plane.
                        ot = opool.tile([128, 2, NPIX], BF16, tag="ot")
                        m1 = tpool.tile([128, NPIX], F16, tag="m1")
                        m2 = tpool.tile([128, NPIX], F16, tag="m2")
                        s = tpool.tile([128, NPIX], F16, tag="s")
                        dd = tpool.tile([128, NPIX], F16, tag="d")
                        nc.scalar.copy(m1[:], pts[1][:])
                        nc.scalar.copy(m2[:], pts[2][:])
                        nc.vector.tensor_add(s[:], m1[:], m2[:])
                        nc.vector.tensor_sub(dd[:], m1[:], m2[:])
                        nc.vector.tensor_add(ot[:, 0], pts[0][:], s[:])
                        nc.vector.tensor_add(ot[:, 1], pts[3][:], dd[:])
                        co = c * 128
                        ha, hb = h0 * JT, (h0 + RCH) * JT
                        last = chunk_id == nchunks - 1
                        if not last:
                            if chunk_id < 18:
                                ring = (nc.sync, nc.scalar, nc.gpsimd)[chunk_id % 3]
                            else:
                                ring = nc.sync if chunk_id % 2 == 0 else nc.scalar
                            ring.dma_start(out=o_h[n, co : co + 128, :, ha:hb], in_=ot[:])
                        else:
                            # drain the final chunk on 3 rings in parallel
                            hm = (ha + hb) // 2
                            nc.sync.dma_start(
                                out=o_h[n, co : co + 128, 0, ha:hb], in_=ot[:, 0]
                            )
                            nc.scalar.dma_start(
                                out=o_h[n, co : co + 128, 1, ha:hm], in_=ot[:, 1, : hm - ha]
                            )
                            nc.gpsimd.dma_start(
                                out=o_h[n, co : co + 128, 1, hm:hb], in_=ot[:, 1, hm - ha :]
                            )
                        for nn, q in dve_v.get(chunk_id, ()):
                            vplane(nc.vector, nn, q, 0, 56)
                        chunk_id += 1
    nc.finalize()
    return nc


_NC_CACHE = None


def _get_nc():
    global _NC_CACHE
    if _NC_CACHE is None:
        _NC_CACHE = build_nc()
    return _NC_CACHE


def _prep_in_maps(x, weight):
    bf16 = ml_dtypes.bfloat16
    w = weight.astype(np.float32)
    g0, g1, g2 = w[:, :, :, 0], w[:, :, :, 1], w[:, :, :, 2]  # [COUT, CIN, KH]
    # q3 negated: PSUM bank 3 accumulates -M3 so y_odd = M1-M2+(bank3)
    Wq = np.stack([g0, (g0 + g1 + g2) * 0.5, (g0 - g1 + g2) * 0.5, -g2], axis=0)
    # layout [CIN, ct, q, kh, m] -> [CIN, 3072]
    Wt = np.zeros((CIN, CTILES, 4, KH, 128), np.float32)
    for ct in range(CTILES):
        Wt[:, ct] = Wq[:, ct * 128 : (ct + 1) * 128].transpose(2, 0, 3, 1)
    w_t = np.ascontiguousarray(Wt.reshape(CIN, WCOLS)).astype(bf16)
    # x planes: A=x[0::2](27), B=x[1::2](27), C=x[2::2](27), D=x[3::2](27)
    P = np.stack(
        [x[:, :, :, 0:54:2], x[:, :, :, 1:55:2], x[:, :, :, 2:56:2], x[:, :, :, 3:56:2]],
        axis=2,
    ).reshape(N, CIN, 4, H * JT)  # [N, CIN, 4, H*27]
    in_maps = []
    for i in range(NCORES):
        xs = np.ascontiguousarray(P[i * NPER : (i + 1) * NPER]).astype(bf16)
        in_maps.append({"x": xs, "w": w_t})
    return in_maps


def run(x, weight, bias, trace=False):
    nc = _get_nc()
    in_maps = _prep_in_maps(x, weight)
    res = run_bass_kernel_spmd(nc, in_maps, core_ids=list(range(NCORES)), trace=trace)
    o = np.concatenate([r["out"] for r in res.results], axis=0).reshape(
        N, COUT, 2, HO, JT
    )
    out = np.empty((N, COUT, HO, WO), np.float32)
    out[:, :, :, 0::2] = o[:, :, 0].astype(np.float32)
    out[:, :, :, 1::2] = o[:, :, 1].astype(np.float32)
    out += np.asarray(bias, np.float32)[None, :, None, None]
    return out, res


def kernel(x: np.ndarray, weight: np.ndarray, bias: np.ndarray) -> np.ndarray:
    out, _ = run(x, weight, bias, trace=False)
    return out.astype(np.float32)
